# revision 23
# baseline (speedup 1.0000x reference)
"""Multi-head attention layer (B=4, L=48*48=2304, C=512, nh=8, dh=64) on 8 TRN2 cores.

Sharding: core c -> (b = c//2, query-half = c%2). Each core computes K/V for all
2304 tokens of its batch, Q for its 1152-token half, full attention for all 8
heads over its queries, and the output projection + residual for its tokens.
Outputs are disjoint row-slices of the final tensor -> no collectives needed.

Dataflow is fully "transposed" ([channels, tokens] layouts) so the PE never
needs a transpose:
  - host ships xT = x[b].T; Wqkv/Wo natural [c_in, c_out] layout serves as lhsT
  - qT/kT: psum[m_chunk, tok] = sum_cc W[cc, m].T @ xT[cc, tok]
  - scoresT[key, qry] = kT_h.T @ qT_h   (1/sqrt(dh) folded into Wq host-side)
  - exp on ScalarE reading 6 psum banks per instruction
  - attn_outT[d, qry] (+ sumexp rows) = V_chunk.T @ expT_chunk, head pair packed
    via col-tiling at partitions 0-63 / 64-127; sums at partitions 0 / 64
  - normalize via DVE reciprocal + DMA partition-broadcast + DVE multiply
  - outT[c_out, tok] = Wo[cc, m].T @ attnT[cc, tok] + x.T + bo residual epilogue
"""

import ml_dtypes
import numpy as np

import concourse.bass as bass
import concourse.tile as tile
from concourse import bacc, mybir
from concourse.bass_utils import run_bass_kernel_spmd

F32 = mybir.dt.float32
BF16 = mybir.dt.bfloat16

B = 4
HW = 48
C = 512
L = HW * HW            # 2304 tokens
NH = 8                 # heads
DH = C // NH           # 64
NCORES = 8
LQ = L // 2            # 1152 queries per core
NCC = C // 128         # 4 contraction chunks of 128 channels
NKC = L // 128         # 18 key chunks of 128
QN = 384               # query tile (free dim of scores/PV matmuls)
NQT = LQ // QN         # 3 query tiles per core
NPAIR = NH // 2        # 4 head pairs
NG = NKC // 3          # 6 groups of 3 key-chunk-pairs (6 psum banks per group)
TTH = L // 3           # 768-token thirds for streaming xT


def build_ir(nc: bass.Bass) -> None:
    xT = nc.dram_tensor("xT", [C, L], BF16, kind="ExternalInput").ap()
    xq = nc.dram_tensor("xq", [C, LQ], BF16, kind="ExternalInput").ap()
    xr = nc.dram_tensor("xr", [C, LQ], F32, kind="ExternalInput").ap()
    wqkv = nc.dram_tensor("wqkv", [C, 3 * C], BF16, kind="ExternalInput").ap()
    wo = nc.dram_tensor("wo", [C, C], BF16, kind="ExternalInput").ap()
    bq = nc.dram_tensor("bq", [128, NCC], F32, kind="ExternalInput").ap()
    bk = nc.dram_tensor("bk", [128, NCC], F32, kind="ExternalInput").ap()
    bv = nc.dram_tensor("bv", [1, C], BF16, kind="ExternalInput").ap()
    bo = nc.dram_tensor("bo", [128, NCC], F32, kind="ExternalInput").ap()
    outT = nc.dram_tensor("outT", [C, LQ], F32, kind="ExternalOutput").ap()
    outT_r = outT.rearrange("(mc p) t -> p mc t", p=128)
    xT_r = xT.rearrange("(cc p) t -> p cc t", p=128)
    xq_r = xq.rearrange("(cc p) t -> p cc t", p=128)
    xr_r = xr.rearrange("(cc p) t -> p cc t", p=128)
    wqkv_r = wqkv.rearrange("(cc p) n -> p cc n", p=128)
    wo_r = wo.rearrange("(cc p) n -> p cc n", p=128)

    with tile.TileContext(nc) as tc:
        with (
            tc.tile_pool(name="const", bufs=1) as cpool,
            tc.tile_pool(name="persist", bufs=1) as pp,
            tc.tile_pool(name="work", bufs=3) as work,
            tc.tile_pool(name="psum", bufs=1, space="PSUM") as psum,
        ):
            # ---- constants
            bq_sb = cpool.tile([128, NCC], F32)
            nc.sync.dma_start(bq_sb[:], bq)
            bk_sb = cpool.tile([128, NCC], F32)
            nc.sync.dma_start(bk_sb[:], bk)
            bo_sb = cpool.tile([128, NCC], F32)
            nc.sync.dma_start(bo_sb[:], bo)
            bv_sb = cpool.tile([1, C], BF16)
            nc.sync.dma_start(bv_sb[:], bv)
            ones_row = cpool.tile([1, 128], BF16)
            nc.vector.memset(ones_row[:], 1.0)
            ones_col = cpool.tile([128, 1], BF16)
            nc.vector.memset(ones_col[:], 1.0)
            ones64 = cpool.tile([128, 64], BF16)
            nc.vector.memset(ones64[:], 1.0)

            # ---- persistent intermediates
            xq_sb = pp.tile([128, NCC, LQ], BF16)
            for cc in range(NCC):
                nc.sync.dma_start(xq_sb[:, cc, :], xq_r[:, cc, :])
            qT_sb = pp.tile([128, NPAIR, LQ], BF16)
            kT_sb = pp.tile([128, NPAIR, L], BF16)
            v_sb = pp.tile([128, NPAIR, NKC, 2 * DH + 1], BF16)
            # ones column at slot DH of every (pair, chunk) fuses head A's
            # sumexp into its PV matmul (lhsT = [V_A | ones], M=65): memset the
            # whole tile to 1.0; the V copies overwrite the data columns.
            nc.vector.memset(v_sb[:, :, :, :], 1.0)
            attnT_sb = pp.tile([128, NCC, LQ], BF16)

            # ---- psum: 6 rotating banks + PV accumulator + sumexp accumulator
            ps_s = psum.tile([128, 6, 512], F32)
            ps_pv = psum.tile([128, 512], F32)
            ps_sum = psum.tile([128, 512], F32)
            bank = 0

            # ================= phase 1 + 2 interleaved =================
            # V first (it gates all attention); then per head-pair: kT/qT for
            # that pair followed by its attention, so projections for pair p+1
            # fill PE slack while pair p's ACT-bound attention runs.
            wqkv_sb = pp.tile([128, NCC, 3 * C], BF16)
            for cc in range(NCC):
                nc.sync.dma_start(wqkv_sb[:, cc, :], wqkv_r[:, cc, :])
            xT_sb = pp.tile([128, NCC, L], BF16)
            for cc in range(NCC):
                nc.sync.dma_start(xT_sb[:, cc, :], xT_r[:, cc, :])

            def emit_v_chunk(tch):
                # V in [token, channel] layout: V[t,n] = xT[cc,t].T @ Wv + bv
                pb = ps_s[:, bank_box[0] % 6, 0:512]
                bank_box[0] += 1
                for cc in range(NCC):
                    nc.tensor.matmul(
                        pb,
                        xT_sb[:, cc, tch * 128 : (tch + 1) * 128],
                        wqkv_sb[:, cc, 2 * C : 3 * C],
                        start=(cc == 0),
                        stop=False,
                    )
                # bias via rank-1 accumulate: ones[1,128].T @ bv[1,512]
                nc.tensor.matmul(
                    pb, ones_row[0:1, 0:128], bv_sb[0:1, :], start=False, stop=True
                )
                pb_h = pb.rearrange("p (pr two d) -> p pr two d", two=2, d=DH)
                nc.vector.tensor_copy(
                    v_sb[:, 0:NPAIR, tch, 0:DH], pb_h[:, :, 0, :]
                )
                nc.vector.tensor_copy(
                    v_sb[:, 0:NPAIR, tch, DH + 1 : 2 * DH + 1], pb_h[:, :, 1, :]
                )

            def emit_kT_tile(m, g):
                # kT chunk m, key-token tile g (keys [g*QN, (g+1)*QN))
                t0 = g * QN
                pb = ps_s[:, bank_box[0] % 6, 0:QN]
                bank_box[0] += 1
                for cc in range(NCC):
                    nc.tensor.matmul(
                        pb,
                        wqkv_sb[:, cc, C + m * 128 : C + (m + 1) * 128],
                        xT_sb[:, cc, t0 : t0 + QN],
                        start=(cc == 0),
                        stop=(cc == NCC - 1),
                    )
                nc.vector.tensor_scalar_add(
                    kT_sb[:, m, t0 : t0 + QN], pb, bk_sb[:, m : m + 1]
                )

            def emit_qT_tile(m, qt):
                t0 = qt * QN
                pb = ps_s[:, bank_box[0] % 6, 0:QN]
                bank_box[0] += 1
                for cc in range(NCC):
                    nc.tensor.matmul(
                        pb,
                        wqkv_sb[:, cc, m * 128 : (m + 1) * 128],
                        xq_sb[:, cc, t0 : t0 + QN],
                        start=(cc == 0),
                        stop=(cc == NCC - 1),
                    )
                nc.vector.tensor_scalar_add(
                    qT_sb[:, m, t0 : t0 + QN], pb, bq_sb[:, m : m + 1]
                )

            bank_box = [bank]

            def emit_norm(p, q0):
                # Denominators: head A at ps_pv row 64 (fused into its PV
                # matmul), head B at ps_sum row 32. Fast-approx reciprocals;
                # broadcast via rank-1 matmuls: 1/denomA -> ps_sum rows 0-63,
                # 1/denomB -> ps_pv rows 64-127 (overwriting the sum rows
                # after they were read); then copy to SBUF and multiply.
                recip_t = work.tile([128, QN], F32, tag="recip")
                nc.vector.reciprocal_approx_fast(
                    recip_t[64:65, :], ps_pv[64:65, 0:QN]
                )
                nc.vector.reciprocal_approx_fast(
                    recip_t[32:33, :], ps_sum[32:33, 0:QN]
                )
                recip_bf = work.tile([128, QN], BF16, tag="recipbf")
                nc.vector.tensor_copy(recip_bf[32:33, :], recip_t[32:33, :])
                nc.vector.tensor_copy(recip_bf[64:65, :], recip_t[64:65, :])
                nc.tensor.matmul(
                    ps_sum[0:64, 0:QN],
                    ones64[64:65, 0:64],
                    recip_bf[64:65, 0:QN],
                )
                nc.tensor.matmul(
                    ps_pv[64:128, 0:QN],
                    ones64[32:33, 0:64],
                    recip_bf[32:33, 0:QN],
                )
                bcast_t = work.tile([128, QN], F32, tag="bcast")
                nc.vector.tensor_copy(bcast_t[0:64, :], ps_sum[0:64, 0:QN])
                nc.vector.tensor_copy(bcast_t[64:128, :], ps_pv[64:128, 0:QN])
                nc.vector.tensor_mul(
                    attnT_sb[0:64, p, q0 : q0 + QN],
                    ps_pv[0:64, 0:QN],
                    bcast_t[0:64, :],
                )
                nc.vector.tensor_mul(
                    attnT_sb[64:128, p, q0 : q0 + QN],
                    ps_sum[64:128, 0:QN],
                    bcast_t[64:128, :],
                )

            def emit_scores(p, q0, g):
                # scoresT[key, qry]; heads A/B in PE row-tiles.
                # A chunks -> banks 0-2, B chunks -> banks 3-5, so exp splits
                # into two 3-bank ACTs.
                for j in range(3):
                    kc = g * 3 + j
                    ks = slice(kc * 128, (kc + 1) * 128)
                    nc.tensor.matmul(
                        ps_s[:, j, 0:QN],
                        kT_sb[0:64, p, ks],
                        qT_sb[0:64, p, q0 : q0 + QN],
                    )
                    nc.tensor.matmul(
                        ps_s[:, 3 + j, 0:QN],
                        kT_sb[64:128, p, ks],
                        qT_sb[64:128, p, q0 : q0 + QN],
                    )

            def emit_exp(exp_t):
                nc.scalar.activation(
                    exp_t[:, 0:3, :],
                    ps_s[:, 0:3, 0:QN],
                    mybir.ActivationFunctionType.Exp,
                )
                nc.scalar.activation(
                    exp_t[:, 3:6, :],
                    ps_s[:, 3:6, 0:QN],
                    mybir.ActivationFunctionType.Exp,
                )

            def emit_pv(p, g, exp_t):
                # attn_outT accumulation. Head A: fused [V_A | ones] lhsT
                # (M=65) -> PV rows 0-63 + sumexp row 64 of ps_pv. Head B:
                # PV at rows 64-127 of ps_sum (col-tile (0,64)) + sumexp at
                # ps_sum row 32 (col-tile (0,32)).
                for j in range(3):
                    kc = g * 3 + j
                    st, sp = (kc == 0), (kc == NKC - 1)
                    nc.tensor.matmul(
                        ps_pv[0:64, 0:QN],
                        v_sb[:, p, kc, 0:DH],
                        exp_t[:, j, :],
                        start=st,
                        stop=sp,
                    )
                    nc.tensor.matmul(
                        ps_pv[64:65, 0:QN],
                        ones_col[:, 0:1],
                        exp_t[:, j, :],
                        start=st,
                        stop=sp,
                    )
                    nc.tensor.matmul(
                        ps_sum[64:128, 0:QN],
                        v_sb[:, p, kc, DH + 1 : 2 * DH + 1],
                        exp_t[:, 3 + j, :],
                        start=st,
                        stop=sp,
                    )
                    nc.tensor.matmul(
                        ps_sum[32:33, 0:QN],
                        ones_col[:, 0:1],
                        exp_t[:, 3 + j, :],
                        start=st,
                        stop=sp,
                    )

            # Software pipeline over all (pair, qtile, group) tiles: PV for
            # group t-1 is emitted after the scores of group t, so the PE has
            # independent work while the ACT exps group t's scores. The V and
            # kT/qT projections are streamed just-in-time into the pipeline:
            # pair 0's projections feed its own first groups; pair p+1's
            # projections ride along pair p's last qtile.
            def prelude(p, qt, g):
                if p == 0 and qt == 0:
                    if g == 0:
                        emit_qT_tile(0, 0)
                    emit_kT_tile(0, g)
                    for j in range(3):
                        emit_v_chunk(3 * g + j)
                elif p == 0 and g == 0:
                    emit_qT_tile(0, qt)
                if qt == NQT - 1 and p < NPAIR - 1:
                    if g < NQT:
                        emit_qT_tile(p + 1, g)
                    emit_kT_tile(p + 1, g)

            groups = [
                (p, qt, g)
                for p in range(NPAIR)
                for qt in range(NQT)
                for g in range(NG)
            ]
            prev = None
            for p, qt, g in groups:
                q0 = qt * QN
                prelude(p, qt, g)
                emit_scores(p, q0, g)
                if prev is not None:
                    pp_, pq0, pg, pexp = prev
                    emit_pv(pp_, pg, pexp)
                    if pg == NG - 1:
                        emit_norm(pp_, pq0)
                exp_t = work.tile([128, 6, QN], BF16, tag="expT")
                emit_exp(exp_t)
                prev = (p, q0, g, exp_t)
            pp_, pq0, pg, pexp = prev
            emit_pv(pp_, pg, pexp)
            emit_norm(pp_, pq0)

            # ================= phase 3: output projection + residual =================
            wo_sb = pp.tile([128, NCC, C], BF16)
            for cc in range(NCC):
                nc.sync.dma_start(wo_sb[:, cc, :], wo_r[:, cc, :])
            xr_sb = pp.tile([128, NCC, LQ], F32)
            for cc in range(NCC):
                nc.sync.dma_start(xr_sb[:, cc, :], xr_r[:, cc, :])
            bank = bank_box[0]
            for m in range(NCC):
                for t0 in range(0, LQ, QN):
                    pb = ps_s[:, bank % 6, 0:QN]
                    bank += 1
                    for cc in range(NCC):
                        nc.tensor.matmul(
                            pb,
                            wo_sb[:, cc, m * 128 : (m + 1) * 128],
                            attnT_sb[:, cc, t0 : t0 + QN],
                            start=(cc == 0),
                            stop=(cc == NCC - 1),
                        )
                    ot = work.tile([128, QN], F32, tag="out")
                    nc.vector.scalar_tensor_tensor(
                        ot[:],
                        pb,
                        bo_sb[:, m : m + 1],
                        xr_sb[:, m, t0 : t0 + QN],
                        op0=mybir.AluOpType.add,
                        op1=mybir.AluOpType.add,
                    )
                    nc.sync.dma_start(outT_r[:, m, t0 : t0 + QN], ot[:])


_compiled = None


def _get_compiled():
    global _compiled
    if _compiled is None:
        nc = bacc.Bacc(
            "TRN2", target_bir_lowering=False, debug=False, num_devices=NCORES
        )
        build_ir(nc)
        nc.compile()
        _compiled = nc
    return _compiled


def make_in_maps(x, Wqkv, bqkv, Wo, bo):
    x = np.asarray(x, np.float32)
    Wqkv = np.asarray(Wqkv, np.float32)
    bqkv = np.asarray(bqkv, np.float32)
    Wo = np.asarray(Wo, np.float32)
    bo = np.asarray(bo, np.float32)

    BF = ml_dtypes.bfloat16
    wqkv_mod = Wqkv.copy()
    wqkv_mod[:, :C] *= 1.0 / np.sqrt(DH)  # fold attention scale into Wq
    wqkv_mod = np.ascontiguousarray(wqkv_mod.astype(BF))
    bq_h = np.ascontiguousarray((bqkv[:C] / np.sqrt(DH)).reshape(NCC, 128).T)
    bk_h = np.ascontiguousarray(bqkv[C : 2 * C].reshape(NCC, 128).T)
    bv_h = np.ascontiguousarray(bqkv[2 * C :][None, :].astype(BF))
    bo_h = np.ascontiguousarray(bo.reshape(NCC, 128).T)
    wo_c = np.ascontiguousarray(Wo.astype(BF))

    in_maps = []
    for c in range(NCORES):
        b, half = c // 2, c % 2
        xb = x[b].reshape(L, C)
        xTb = np.ascontiguousarray(xb.T)
        xTb_bf = xTb.astype(BF)
        sl = slice(half * LQ, (half + 1) * LQ)
        in_maps.append(
            {
                "xT": np.ascontiguousarray(xTb_bf),
                "xq": np.ascontiguousarray(xTb_bf[:, sl]),
                "xr": np.ascontiguousarray(xTb[:, sl]),
                "wqkv": wqkv_mod,
                "wo": wo_c,
                "bq": bq_h,
                "bk": bk_h,
                "bv": bv_h,
                "bo": bo_h,
            }
        )
    return in_maps


def assemble_output(results):
    out = np.empty((B, L, C), np.float32)
    for c in range(NCORES):
        b, half = c // 2, c % 2
        out[b, half * LQ : (half + 1) * LQ, :] = results[c]["outT"].T
    return out.reshape(B, HW, HW, C)


def kernel(x, Wqkv, bqkv, Wo, bo):
    nc = _get_compiled()
    in_maps = make_in_maps(x, Wqkv, bqkv, Wo, bo)
    res = run_bass_kernel_spmd(nc, in_maps, list(range(NCORES)))
    return assemble_output(res.results)
